# revision 78
# baseline (speedup 1.0000x reference)
"""Trainium2 Bass kernel for nn_MultiHeadAttention_46093589021200.

Causal MHA: B=4, S=2048, E=1024, H=16, D=64, with the reference's
"no-transpose-back" reshape (b,h,s,d)->(b,s,e) before the output projection.

Sharding: pure head-parallel, 2 heads per core, zero collectives.
Because of the reshape quirk, output rows s' in [h*128,(h+1)*128) depend only
on head h, so each core produces two independent 128-row output bands per
batch.

Device algorithm (per core, fp16 compute / fp32 PSUM accumulation):
  - qkvT = Wqkv_c^T @ x^T computed directly in head-major [col, s] layout
    (x is passed pre-transposed+pre-cast from the host; contraction over e
    in 8 PSUM-accumulated K=128 chunks; v's bias-add drains on ACT, q/k on
    DVE).
  - v transposed to [s, d] via the DMA xbar (one batched transpose per
    (head, 512-col chunk)), augmented with a ones column per head so the PV
    matmul also produces softmax denominators.
  - scoresT[k,q] per 128-k chunk on PE, two heads packed into row groups
    0-1 / 2-3 of the systolic array (K=64 each).
  - exp on ACT, one instruction covering both heads per chunk
    (scale=1/sqrt(D) folded in); causality = skipping k>q chunks entirely
    plus a triangular fp16 mask multiply on diagonal chunks (both heads in
    one DVE op via a doubled mask).
  - PV in q-major form: att_q[q, d_aug] accumulates with the exp'd score
    chunk as the stationary operand and v_aug streaming (N=65 per matmul,
    half the streaming cost of d-major PV); rowsums land per-partition so
    the normalize is one PSUM->SBUF staging copy + reciprocal +
    tensor_scalar multiplies (no partition broadcast).
  - attn transposed back to [h*64+d, q] per 512-block via the DMA xbar,
    then copied into attB: partitions 0-63 straight, 64-127 shifted one
    column left, so o_proj contracts K=128 (w-pairs) with one stride-16 AP.
  - o_proj: per 128-row band, 8 K=128-chunk matmuls + a K=1 ones-row
    matmul for the bias; output stored fp16 (host casts to fp32).

Scheduling: qkv n-chunks, o_proj(b-1) groups, and attention gq-blocks are
interleaved so the ACT engine's exp stream (the local bottleneck in the
attention phase) stays hidden under PE work; filler placement follows the
4-buffer PSUM ring so no matmul group parks on an att_ps free.  All
finish (transpose+copy) chains carry nosync deps on the next v2-transpose
pair — the scheduler would otherwise hoist them and the transpose-vs-DMA
serialization guard would stall PV.  x loads ride the SWDGE (gpsimd) ring
for the same reason.

NOTES: column-positioned matmuls (tile_position=(0,32j)) and GPSIMD
PSUM reads mis-execute / fail verification on this hardware path even
though the cost model accepts them; PSUM accumulation groups are
per-bank, so column-sliced accumulation windows must share one
start/stop.
"""

import sys

if "/opt/trn_rl_repo" not in sys.path:
    sys.path.insert(0, "/opt/trn_rl_repo")

import numpy as np

B, S, E, H = 4, 2048, 1024, 16
D = E // H          # 64
NCORES = 8
HPC = H // NCORES   # heads per core = 2
COLS = 3 * HPC * D  # 384 qkv columns per core
SCALE = 1.0 / float(np.sqrt(D))

_CACHE = {}


def _build_program():
    import concourse.bass as bass  # noqa: F401
    import concourse.tile as tile
    from concourse import bacc, mybir
    from concourse.instruction_name_ordered_set import InstructionNameOrderedSet

    f16 = mybir.dt.float16
    f32 = mybir.dt.float32
    Exp = mybir.ActivationFunctionType.Exp

    nc = bacc.Bacc("TRN2", target_bir_lowering=False, debug=False)

    xT = nc.dram_tensor("xT", [B, E, S], f16, kind="ExternalInput")
    wqkv = nc.dram_tensor("wqkv", [E, COLS], f16, kind="ExternalInput")
    bqkv = nc.dram_tensor("bqkv", [128, 3], f32, kind="ExternalInput")
    wo8 = nc.dram_tensor("wo8", [8, 128, E], f16, kind="ExternalInput")
    bo1 = nc.dram_tensor("bo1", [1, E], f16, kind="ExternalInput")
    trimask = nc.dram_tensor("trimask", [128, 256], f16, kind="ExternalInput")
    out = nc.dram_tensor("out", [B, HPC, 128, E], f16, kind="ExternalOutput")

    with tile.TileContext(nc) as tc:
        with (
            tc.tile_pool(name="const", bufs=1) as cp,
            tc.tile_pool(name="sb", bufs=2) as sb,
            tc.tile_pool(name="sb3", bufs=3) as sb3,
            tc.tile_pool(name="ps", bufs=2, space="PSUM") as ps,
        ):
            # ---- constants resident in SBUF for the whole kernel ----
            # critical path (SP ring): wqkv, then batch-0 x slices
            wqkv_sb = cp.tile([128, 8 * COLS], f16)   # [p, ec*384+col]
            nc.sync.dma_start(
                wqkv_sb.rearrange("p (ec c) -> p ec c", ec=8),
                wqkv.ap().rearrange("(ec p) c -> p ec c", p=128),
            )
            # non-critical constants on the ACT HWDGE ring
            bqkv_sb = cp.tile([128, 3], f32)
            nc.scalar.dma_start(bqkv_sb, bqkv.ap())
            trimask_sb = cp.tile([128, 256], f16)  # two copies side by side
            nc.scalar.dma_start(trimask_sb, trimask.ap())
            bo_sb = cp.tile([1, E], f16)
            nc.scalar.dma_start(bo_sb, bo1.ap())
            # wo8 loads in per-j chunks, interleaved into batch 0's stream:
            # a long const DMA in flight would stall the transpose-vs-DMA
            # serialization guard ahead of the v2 transposes
            wo8_sb = cp.tile([128, 8 * E], f16)       # [p, j*1024+c]

            def emit_wo8(j0, j1):
                for j in range(j0, j1):
                    nc.scalar.dma_start(
                        wo8_sb[:, j * E : (j + 1) * E], wo8.ap()[j]
                    )

            ones_sb = cp.tile([1, 128], f16)
            nc.vector.memset(ones_sb, 1.0)

            def emit_xload(b, lo=0, hi=4, xt_sb=None):
                # b==0: n-chunk slices, staggered around vt0 so the
                # transpose guard only waits on slice 0
                if xt_sb is None:
                    xt_sb = sb.tile([128, 8 * S], f16, tag="xt")
                xt3 = xt_sb.rearrange("p (ec s) -> p ec s", ec=8)
                xd3 = xT.ap()[b].rearrange("(ec p) s -> p ec s", p=128)
                # on the SWDGE (gpsimd) ring: HWDGE x-load transfers would
                # make the transpose-serialization guard stall v2 transposes
                if b == 0:
                    for n in range(lo, hi):
                        nc.gpsimd.dma_start(
                            xt3[:, :, n * 512 : (n + 1) * 512],
                            xd3[:, :, n * 512 : (n + 1) * 512],
                        )
                else:
                    for ec in range(8):
                        nc.gpsimd.dma_start(xt3[:, ec], xd3[:, ec])
                return xt_sb

            def emit_qkv_mgroup(xt_sb, qkvT2_sb, n, m):
                # one 512-wide s-chunk of one of q/k/v:
                # m=0 -> [q_h0|q_h1], m=1 -> [k_h0|k_h1], m=2 -> v
                pq = ps.tile([128, 512], f32, tag="acc", name="pq", bufs=4)
                for ec in range(8):
                    nc.tensor.matmul(
                        pq,
                        wqkv_sb[:, ec * COLS + m * 128
                                : ec * COLS + (m + 1) * 128],
                        xt_sb[:, ec * S + n * 512 : ec * S + (n + 1) * 512],
                        start=(ec == 0),
                        stop=(ec == 7),
                    )
                # bias-add + PSUM->SBUF drain.  GPSIMD cannot read PSUM on
                # hardware.  Early v groups (n<=1) drain on ACT (idle during
                # the qkv head, keeps DVE short for the normalize chain);
                # later ones stay on DVE so they don't delay mid-attention
                # exps on ACT.
                dst = qkvT2_sb[:, m * S + n * 512 : m * S + (n + 1) * 512]
                if m == 2 and n <= 1:
                    nc.scalar.activation(
                        dst, pq, mybir.ActivationFunctionType.Identity,
                        bias=bqkv_sb[:, m : m + 1],
                    )
                else:
                    nc.vector.tensor_scalar_add(dst, pq, bqkv_sb[:, m : m + 1])

            def emit_vtrans(qkvT2_sb, v2v, n):
                # transpose vT2 [d2, s] -> [s, d] batched per (h, n-chunk)
                insts = []
                for h in range(2):
                    insts.append(nc.sync.dma_start(
                        v2v[:, 4 * n : 4 * n + 4, h * 80 : h * 80 + 64],
                        qkvT2_sb[h * 64 : (h + 1) * 64,
                                 2 * S + n * 512 : 2 * S + (n + 1) * 512],
                        transpose=True,
                    ))
                return insts

            def emit_attn_gq(qkvT2_sb, v2_sb, attQ_sb, attT_sb, attB, gq,
                             fillers=()):
                fillers = dict(fillers)
                njk = 4 * gq + 4
                # q-major PV accumulators: [q=128, sub*65 + (d|rowsum)],
                # one per head.  N=65 per PV matmul (vs W) halves the PE
                # streaming cost; rowsums land per-partition so the
                # normalize is reciprocal + tensor_scalar (no broadcast).
                att_ps = [
                    ps.tile([128, 260], f32, tag="acc",
                            name=f"att{h}_ps", bufs=4)
                    for h in range(2)
                ]
                for kj in range(njk):
                    if kj in fillers:
                        fillers[kj]()
                    q_lo = max(gq * 512, kj * 128)
                    W = gq * 512 + 512 - q_lo
                    qo = q_lo - gq * 512
                    sc_ps = ps.tile([128, 1024], f32, tag="scores",
                                    name="sc_ps")
                    ex_sb = sb3.tile([128, 1024], f16, tag="expT",
                                     name="ex_sb")
                    for h in range(2):
                        # scoresT[k, q] = kT-chunk contracted with qT
                        nc.tensor.matmul(
                            sc_ps[:, h * 512 + qo : h * 512 + qo + W],
                            qkvT2_sb[h * 64 : (h + 1) * 64,
                                     S + kj * 128 : S + (kj + 1) * 128],
                            qkvT2_sb[h * 64 : (h + 1) * 64, q_lo : q_lo + W],
                            start=True,
                            stop=True,
                            tile_position=(h * 64, 0),
                        )
                    # exp over both heads in one ACT instruction; the first
                    # chunk of each block is split in half so its first PV
                    # sub-chunks start ~0.5 us sooner (pipeline fill)
                    exv = ex_sb.rearrange("p (h q) -> p h q", h=2)
                    scv = sc_ps.rearrange("p (h q) -> p h q", h=2)
                    if kj == 0:
                        nc.scalar.activation(
                            exv[:, :, 0:256], scv[:, :, 0:256], Exp,
                            scale=SCALE)
                        nc.scalar.activation(
                            exv[:, :, 256:512], scv[:, :, 256:512], Exp,
                            scale=SCALE)
                    else:
                        nc.scalar.activation(
                            exv[:, :, qo : qo + W], scv[:, :, qo : qo + W],
                            Exp, scale=SCALE)
                    if kj >= 4 * gq:  # diagonal chunk: zero out k > q
                        exv = ex_sb.rearrange("p (h q) -> p h q", h=2)
                        nc.vector.tensor_mul(
                            exv[:, :, qo : qo + 128],
                            exv[:, :, qo : qo + 128],
                            trimask_sb.rearrange("p (h q) -> p h q", h=2),
                        )
                    # att_q[q, d_aug] += ex[k, q]^T-contracted with v_aug
                    # (ex chunk is the stationary; LDWEIGHTS overlaps).
                    # One accumulation group per tile: PSUM groups are
                    # per-bank, so the sub-chunk column slices share a
                    # single start/stop window.
                    for h in range(2):
                        for c in range(max(0, kj - 4 * gq), 4):
                            nc.tensor.matmul(
                                att_ps[h][:, c * 65 : c * 65 + 65],
                                ex_sb[:, h * 512 + c * 128
                                      : h * 512 + (c + 1) * 128],
                                v2_sb[:, kj * 160 + h * 80
                                      : kj * 160 + h * 80 + 65],
                                start=(kj == 0 and c == 0),
                                stop=(kj == njk - 1 and c == 3),
                            )
                # normalize: stage PSUM->SBUF in one copy (frees the 'acc'
                # ring, which gates the next qkv groups, ~1 us sooner), then
                # per-partition reciprocal + scalar muls into attQ
                # [q, c*128 + h*64 + d] fp16
                for h in range(2):
                    st = sb.tile([128, 260], f32, tag=f"st{h}",
                                 name=f"st{h}")
                    nc.vector.tensor_copy(st, att_ps[h])
                    rr4 = sb.tile([128, 4], f32, tag=f"rr{h}", name=f"rr{h}")
                    nc.vector.reciprocal(
                        rr4, st.rearrange("p (c t) -> p c t", t=65)[:, :, 64]
                    )
                    for c in range(4):
                        nc.vector.tensor_scalar_mul(
                            attQ_sb[:, gq * 512 + c * 128 + h * 64
                                    : gq * 512 + c * 128 + h * 64 + 64],
                            st[:, c * 65 : c * 65 + 64],
                            rr4[:, c : c + 1],
                        )
                # (the attQ->attT transpose and attB copies are emitted
                # later via emit_attn_finish, so they never park ahead of
                # the next v2 transpose on the SP ring)

            def emit_attn_finish(attQ_sb, attT_sb, attB, gq, after=()):
                # transpose attQ block -> attT [h*64+d, q] via the DMA xbar.
                # `after`: nosync deps forcing the scheduler to place this
                # chain behind the given v2 transposes on the SP ring (it
                # would otherwise hoist it ahead of them, and the transpose
                # guard would stall PV on v2 for ~8 us).
                attT3 = attT_sb.rearrange("p (c t) -> p c t", t=128)
                ti = nc.sync.dma_start(
                    attT3[:, 4 * gq : 4 * gq + 4, :],
                    attQ_sb[:, gq * 512 : (gq + 1) * 512],
                    transpose=True,
                )
                if after:
                    deps = InstructionNameOrderedSet()
                    for bi in after:
                        deps.add(bi.ins.name)
                    ti.ins.add_nosync_dependencies_from(deps)
                # o_proj operand (attB cols h*S+q): top = attn_h[d, q],
                # bottom = attn_h[d, q+1] (shifted), so K=128 w-pair chunks
                # read with one stride-16 AP.  Block gq enables shifted-dest
                # columns [gq*512-1, gq*512+511).
                lo = gq * 512
                src_lo = max(1, lo)
                for h in range(2):
                    nc.sync.dma_start(
                        attB[0:64, h * S + lo : h * S + lo + 512],
                        attT_sb[h * 64 : (h + 1) * 64, lo : lo + 512],
                    )
                    nc.sync.dma_start(
                        attB[64:128, h * S + src_lo - 1 : h * S + lo + 511],
                        attT_sb[h * 64 : (h + 1) * 64, src_lo : lo + 512],
                    )

            def emit_oproj_group(b, attB, out_sbs, h, n2):
                # out_band[u, c] = sum_j sum_{k<128}
                #   attB[k, h*S + u*16+2j] Wo[128j+k, c]   (K=128 per matmul)
                attv = attB[:, h * S : (h + 1) * S].rearrange(
                    "p (u w) -> p w u", w=16)
                po = ps.tile([128, 512], f32, tag="acc", name="po", bufs=4)
                for j in range(8):
                    nc.tensor.matmul(
                        po,
                        attv[:, 2 * j, :],
                        wo8_sb[:, j * E + n2 * 512 : j * E + n2 * 512 + 512],
                        start=(j == 0),
                        stop=False,
                    )
                # bias row via K=1 ones matmul
                nc.tensor.matmul(
                    po,
                    ones_sb[0:1, :],
                    bo_sb[0:1, n2 * 512 : (n2 + 1) * 512],
                    start=False,
                    stop=True,
                )
                nc.vector.tensor_copy(
                    out_sbs[h][:, n2 * 512 : (n2 + 1) * 512], po
                )
                if n2 == 1:
                    nc.sync.dma_start(out.ap()[b, h], out_sbs[h])

            # schedule per batch: [qkv n0, qkv n1, gq0, qkv n2, gq1, qkv n3,
            # gq2, gq3] with o_proj(b-1) groups woven into the ACT-bound
            # gq2/gq3 regions so PE never drains while ACT catches up.
            prev = None
            fin_args = None
            for b in range(B):
                xt_sb = emit_xload(b, 0, 1)
                qkvT2_sb = sb.tile([128, 3 * S], f16, tag="qkvT2")
                # v2 chunk layout (stride 160):
                #   [v_h0(64) | ones | pad15 | v_h1(64) | ones | pad15]
                v2_sb = sb.tile([128, 160 * (S // 128)], f16, tag="v2")
                v2v = v2_sb.rearrange("p (c t) -> p c t", t=160)
                nc.gpsimd.memset(v2v[:, :, 64:65], 1.0)
                nc.gpsimd.memset(v2v[:, :, 144:145], 1.0)
                attQ_sb = sb.tile([128, S], f16, tag="attQ", name="attQ")
                attT_sb = sb.tile([128, S], f16, tag="attT", name="attT")
                attB = sb.tile([128, 2 * S], f16, tag="attB", name="attB")

                vts = {}

                def qkv_m(n, m, vt=False):
                    def fn():
                        emit_qkv_mgroup(xt_sb, qkvT2_sb, n, m)
                        if vt:
                            vts[n] = emit_vtrans(qkvT2_sb, v2v, n)
                    return fn

                def attn(gq, fillers=()):
                    emit_attn_gq(qkvT2_sb, v2_sb, attQ_sb, attT_sb, attB,
                                 gq, fillers)

                def finish(gq, after_n):
                    def fn():
                        emit_attn_finish(attQ_sb, attT_sb, attB, gq,
                                         after=vts.get(after_n, ()))
                    return fn

                # v-group first so the v2 transpose (2.2 us DMA latency)
                # overlaps the q/k groups instead of stalling gq0's PV
                emit_qkv_mgroup(xt_sb, qkvT2_sb, 0, 2)
                vts[0] = emit_vtrans(qkvT2_sb, v2v, 0)
                if b == 0:
                    emit_xload(0, 1, 4, xt_sb)  # rest of batch-0 x after vt0
                if fin_args is not None:
                    # deferred gq2-finish of b-1, ordered behind vt0
                    emit_attn_finish(*fin_args, 2, after=vts[0])
                emit_qkv_mgroup(xt_sb, qkvT2_sb, 0, 0)
                emit_qkv_mgroup(xt_sb, qkvT2_sb, 0, 1)
                emit_qkv_mgroup(xt_sb, qkvT2_sb, 1, 0)
                emit_qkv_mgroup(xt_sb, qkvT2_sb, 1, 1)
                # filler layout follows the 'acc' PSUM ring (4 bufs): at
                # most two pq/po groups inside each attention block (their
                # ring gates resolve pre-block), one right after it, rest in
                # the head.  finish() DMAs allocate no PSUM; each is nosync-
                # ordered behind the next v2-transpose pair so the scheduler
                # never parks its copy chain ahead of them on the SP ring.
                f0 = [(1, qkv_m(1, 2, vt=True)), (2, qkv_m(2, 0))]
                if fin_args is not None:
                    fa = fin_args
                    f0.append((3, lambda: emit_attn_finish(
                        *fa, 3, after=vts[1])))
                attn(0, f0)
                emit_qkv_mgroup(xt_sb, qkvT2_sb, 2, 1)  # post-gq0
                if b == 0:
                    emit_wo8(0, 8)  # ACT ring; emitted after batch 0's
                    # first attention block so the transfers never sit in
                    # flight ahead of the startup v2 transposes
                attn(1, [(2, qkv_m(2, 2, vt=True)), (5, qkv_m(3, 0)),
                         (6, finish(0, 2))])
                emit_qkv_mgroup(xt_sb, qkvT2_sb, 3, 1)  # post-gq1
                if prev is not None:
                    pb, pattB, pout = prev
                    attn(2, [(2, qkv_m(3, 2, vt=True)), (4, finish(1, 3)),
                             (5, lambda: emit_oproj_group(
                                 pb, pattB, pout, 0, 0))])
                    emit_oproj_group(pb, pattB, pout, 0, 1)  # post-gq2
                    attn(3, [(2, lambda: emit_oproj_group(
                                 pb, pattB, pout, 1, 0)),
                             (5, lambda: emit_oproj_group(
                                 pb, pattB, pout, 1, 1))])
                else:
                    attn(2, [(2, qkv_m(3, 2, vt=True)), (4, finish(1, 3))])
                    attn(3)
                out_sbs = [
                    sb.tile([128, E], f16, tag=f"outsb{h}", name=f"out{h}_sb")
                    for h in range(2)
                ]
                prev = (b, attB, out_sbs)
                fin_args = (attQ_sb, attT_sb, attB)
            pb, pattB, pout = prev
            emit_attn_finish(*fin_args, 2)
            emit_attn_finish(*fin_args, 3)
            for h in range(2):
                for n2 in range(2):
                    emit_oproj_group(pb, pattB, pout, h, n2)

    nc.compile()
    return nc


def _get_program(dbg=False):
    key = ("nc",)
    if key not in _CACHE:
        _CACHE[key] = _build_program()
    return _CACHE[key]


def _host_inputs(x, Wqkv, bqkv, Wo, bo):
    """Build per-core input maps (host-side layout prep: cast/slice/transpose)."""
    xT = np.ascontiguousarray(x.transpose(0, 2, 1)).astype(np.float16)

    wo8 = np.ascontiguousarray(
        Wo.astype(np.float16).reshape(8, 128, E)
    )
    bo1 = bo.astype(np.float16)[None, :]

    k_idx = np.arange(128)[:, None]
    q_idx = np.arange(128)[None, :]
    tri = (k_idx <= q_idx).astype(np.float16)
    trimask = np.concatenate([tri, tri], axis=1)  # one copy per head

    in_maps = []
    for c in range(NCORES):
        cols = []
        for off in (0, 64, 128):  # q, k, v
            for h in (HPC * c, HPC * c + 1):
                cols.extend(range(h * 3 * D + off, h * 3 * D + off + 64))
        cols = np.asarray(cols)
        in_maps.append(
            {
                "xT": xT,
                "wqkv": np.ascontiguousarray(Wqkv[:, cols]).astype(np.float16),
                "bqkv": np.ascontiguousarray(
                    bqkv[cols].reshape(3, 128).T
                ).astype(np.float32),
                "wo8": wo8,
                "bo1": bo1,
                "trimask": trimask,
            }
        )
    return in_maps


def kernel(x, mask, Wqkv, bqkv, Wo, bo, _n_cores=NCORES, _trace=False, _dbg=False):
    """Full-input, full-output MHA. `mask` is the causal tril mask (hardcoded)."""
    from concourse.bass_utils import run_bass_kernel_spmd

    nc = _get_program()
    in_maps = _host_inputs(
        np.asarray(x), np.asarray(Wqkv), np.asarray(bqkv), np.asarray(Wo), np.asarray(bo)
    )[:_n_cores]
    res = run_bass_kernel_spmd(
        nc, in_maps, core_ids=list(range(_n_cores)), trace=_trace
    )
    out_full = np.zeros((B, S, E), np.float32)
    for c in range(_n_cores):
        o = res.results[c]["out"]  # [B, HPC, 128, E]
        for h in range(HPC):
            g = HPC * c + h
            out_full[:, g * 128 : (g + 1) * 128, :] = o[:, h]
    _CACHE["last_results"] = res
    return out_full


# revision 91
# speedup vs baseline: 1.0125x; 1.0125x over previous
"""Trainium2 Bass kernel for nn_MultiHeadAttention_46093589021200.

Causal MHA: B=4, S=2048, E=1024, H=16, D=64, with the reference's
"no-transpose-back" reshape (b,h,s,d)->(b,s,e) before the output projection.

Sharding: pure head-parallel, 2 heads per core, zero collectives.
Because of the reshape quirk, output rows s' in [h*128,(h+1)*128) depend only
on head h, so each core produces two independent 128-row output bands per
batch.

Device algorithm (per core, fp16 compute / fp32 PSUM accumulation):
  - qkvT = Wqkv_c^T @ x^T computed directly in head-major [col, s] layout
    (x is passed pre-transposed+pre-cast from the host; contraction over e
    in 8 PSUM-accumulated K=128 chunks; v's bias-add drains on ACT, q/k on
    DVE).
  - v transposed to [s, d] via the DMA xbar (one batched transpose per
    (head, 512-col chunk)), augmented with a ones column per head so the PV
    matmul also produces softmax denominators.
  - scoresT[k,q] per 128-k chunk on PE, two heads packed into row groups
    0-1 / 2-3 of the systolic array (K=64 each).
  - exp on ACT, one instruction covering both heads per chunk
    (scale=1/sqrt(D) folded in); causality = skipping k>q chunks entirely
    plus a triangular fp16 mask multiply on diagonal chunks (both heads in
    one DVE op via a doubled mask).
  - PV in q-major form: att_q[q, d_aug] accumulates with the exp'd score
    chunk as the stationary operand and v_aug streaming (N=65 per matmul,
    half the streaming cost of d-major PV); rowsums land per-partition so
    the normalize is one PSUM->SBUF staging copy + reciprocal +
    tensor_scalar multiplies (no partition broadcast).
  - attn transposed back to [h*64+d, q] per 512-block via the DMA xbar,
    then copied into attB: partitions 0-63 straight, 64-127 shifted one
    column left, so o_proj contracts K=128 (w-pairs) with one stride-16 AP.
  - o_proj: per 128-row band, 8 K=128-chunk matmuls; the bias is folded
    into the DVE PSUM-drain (tensor_add against a host-broadcast bias
    tile); output stored fp16 (host casts to fp32).

Scheduling: qkv n-chunks, o_proj(b-1) groups, and attention gq-blocks are
interleaved so the ACT engine's exp stream (the local bottleneck in the
attention phase) stays hidden under PE work; filler placement follows the
4-buffer PSUM ring so no matmul group parks on an att_ps free.  All
finish (transpose+copy) chains carry nosync deps on the next v2-transpose
pair — the scheduler would otherwise hoist them and the transpose-vs-DMA
serialization guard would stall PV.  x loads ride the SWDGE (gpsimd) ring
for the same reason.

NOTES: column-positioned matmuls (tile_position=(0,32j)) and GPSIMD
PSUM reads mis-execute / fail verification on this hardware path even
though the cost model accepts them; PSUM accumulation groups are
per-bank, so column-sliced accumulation windows must share one
start/stop.
"""

import sys

if "/opt/trn_rl_repo" not in sys.path:
    sys.path.insert(0, "/opt/trn_rl_repo")

import numpy as np

B, S, E, H = 4, 2048, 1024, 16
D = E // H          # 64
NCORES = 8
HPC = H // NCORES   # heads per core = 2
COLS = 3 * HPC * D  # 384 qkv columns per core
SCALE = 1.0 / float(np.sqrt(D))

_CACHE = {}


def _build_program():
    import concourse.bass as bass  # noqa: F401
    import concourse.tile as tile
    from concourse import bacc, mybir
    from concourse.instruction_name_ordered_set import InstructionNameOrderedSet

    f16 = mybir.dt.float16
    f32 = mybir.dt.float32
    Exp = mybir.ActivationFunctionType.Exp

    nc = bacc.Bacc("TRN2", target_bir_lowering=False, debug=False)

    xT = nc.dram_tensor("xT", [B, E, S], f16, kind="ExternalInput")
    wqkv = nc.dram_tensor("wqkv", [E, COLS], f16, kind="ExternalInput")
    bqkv = nc.dram_tensor("bqkv", [128, 3], f32, kind="ExternalInput")
    wo8 = nc.dram_tensor("wo8", [8, 128, E], f16, kind="ExternalInput")
    bo1 = nc.dram_tensor("bo1", [128, E], f16, kind="ExternalInput")
    trimask = nc.dram_tensor("trimask", [128, 256], f16, kind="ExternalInput")
    out = nc.dram_tensor("out", [B, HPC, 128, E], f16, kind="ExternalOutput")

    with tile.TileContext(nc) as tc:
        with (
            tc.tile_pool(name="const", bufs=1) as cp,
            tc.tile_pool(name="sb", bufs=2) as sb,
            tc.tile_pool(name="sb3", bufs=3) as sb3,
            tc.tile_pool(name="ps", bufs=2, space="PSUM") as ps,
        ):
            # ---- constants resident in SBUF for the whole kernel ----
            # critical path (SP ring): wqkv, then batch-0 x slices
            wqkv_sb = cp.tile([128, 8 * COLS], f16)   # [p, ec*384+col]
            nc.sync.dma_start(
                wqkv_sb.rearrange("p (ec c) -> p ec c", ec=8),
                wqkv.ap().rearrange("(ec p) c -> p ec c", p=128),
            )
            # non-critical constants on the ACT HWDGE ring
            bqkv_sb = cp.tile([128, 3], f32)
            nc.scalar.dma_start(bqkv_sb, bqkv.ap())
            trimask_sb = cp.tile([128, 256], f16)  # two copies side by side
            nc.scalar.dma_start(trimask_sb, trimask.ap())
            bo2d_sb = cp.tile([128, E], f16)  # bias broadcast to all rows
            nc.scalar.dma_start(bo2d_sb, bo1.ap())
            # wo8 loads in per-j chunks, interleaved into batch 0's stream:
            # a long const DMA in flight would stall the transpose-vs-DMA
            # serialization guard ahead of the v2 transposes
            wo8_sb = cp.tile([128, 8 * E], f16)       # [p, j*1024+c]

            def emit_wo8(j0, j1):
                for j in range(j0, j1):
                    nc.scalar.dma_start(
                        wo8_sb[:, j * E : (j + 1) * E], wo8.ap()[j]
                    )

            ones_sb = cp.tile([1, 128], f16)
            nc.vector.memset(ones_sb, 1.0)

            def emit_xload(b, lo=0, hi=4, xt_sb=None):
                # b==0: n-chunk slices, staggered around vt0 so the
                # transpose guard only waits on slice 0
                if xt_sb is None:
                    xt_sb = sb.tile([128, 8 * S], f16, tag="xt")
                xt3 = xt_sb.rearrange("p (ec s) -> p ec s", ec=8)
                xd3 = xT.ap()[b].rearrange("(ec p) s -> p ec s", p=128)
                # on the SWDGE (gpsimd) ring: HWDGE x-load transfers would
                # make the transpose-serialization guard stall v2 transposes
                if b == 0:
                    for n in range(lo, hi):
                        nc.gpsimd.dma_start(
                            xt3[:, :, n * 512 : (n + 1) * 512],
                            xd3[:, :, n * 512 : (n + 1) * 512],
                        )
                else:
                    for ec in range(8):
                        nc.gpsimd.dma_start(xt3[:, ec], xd3[:, ec])
                return xt_sb

            def emit_qkv_mgroup(xt_sb, qkvT2_sb, n, m):
                # one 512-wide s-chunk of one of q/k/v:
                # m=0 -> [q_h0|q_h1], m=1 -> [k_h0|k_h1], m=2 -> v
                pq = ps.tile([128, 512], f32, tag="acc", name="pq", bufs=4)
                for ec in range(8):
                    nc.tensor.matmul(
                        pq,
                        wqkv_sb[:, ec * COLS + m * 128
                                : ec * COLS + (m + 1) * 128],
                        xt_sb[:, ec * S + n * 512 : ec * S + (n + 1) * 512],
                        start=(ec == 0),
                        stop=(ec == 7),
                    )
                # bias-add + PSUM->SBUF drain.  GPSIMD cannot read PSUM on
                # hardware.  Early v groups (n<=1) drain on ACT (idle during
                # the qkv head, keeps DVE short for the normalize chain);
                # later ones stay on DVE so they don't delay mid-attention
                # exps on ACT.
                dst = qkvT2_sb[:, m * S + n * 512 : m * S + (n + 1) * 512]
                if m == 2 and n <= 1:
                    nc.scalar.activation(
                        dst, pq, mybir.ActivationFunctionType.Identity,
                        bias=bqkv_sb[:, m : m + 1],
                    )
                else:
                    nc.vector.tensor_scalar_add(dst, pq, bqkv_sb[:, m : m + 1])

            def emit_vtrans(qkvT2_sb, v2v, n):
                # transpose vT2 [d2, s] -> [s, d] batched per (h, n-chunk)
                insts = []
                for h in range(2):
                    insts.append(nc.sync.dma_start(
                        v2v[:, 4 * n : 4 * n + 4, h * 80 : h * 80 + 64],
                        qkvT2_sb[h * 64 : (h + 1) * 64,
                                 2 * S + n * 512 : 2 * S + (n + 1) * 512],
                        transpose=True,
                    ))
                return insts

            def emit_attn_gq(qkvT2_sb, v2_sb, attQ_sb, attT_sb, attB, gq,
                             fillers=()):
                fillers = dict(fillers)
                njk = 4 * gq + 4
                # q-major PV accumulators: [q=128, sub*65 + (d|rowsum)],
                # one per head.  N=65 per PV matmul (vs W) halves the PE
                # streaming cost; rowsums land per-partition so the
                # normalize is reciprocal + tensor_scalar (no broadcast).
                att_ps = [
                    ps.tile([128, 260], f32, tag="acc",
                            name=f"att{h}_ps", bufs=4)
                    for h in range(2)
                ]
                for kj in range(njk):
                    if kj in fillers:
                        fillers[kj]()
                    q_lo = max(gq * 512, kj * 128)
                    W = gq * 512 + 512 - q_lo
                    qo = q_lo - gq * 512
                    sc_ps = ps.tile([128, 1024], f32, tag="scores",
                                    name="sc_ps")
                    ex_sb = sb3.tile([128, 1024], f16, tag="expT",
                                     name="ex_sb")
                    for h in range(2):
                        # scoresT[k, q] = kT-chunk contracted with qT
                        nc.tensor.matmul(
                            sc_ps[:, h * 512 + qo : h * 512 + qo + W],
                            qkvT2_sb[h * 64 : (h + 1) * 64,
                                     S + kj * 128 : S + (kj + 1) * 128],
                            qkvT2_sb[h * 64 : (h + 1) * 64, q_lo : q_lo + W],
                            start=True,
                            stop=True,
                            tile_position=(h * 64, 0),
                        )
                    # exp over both heads in one ACT instruction; the first
                    # chunk of each block is split in half so its first PV
                    # sub-chunks start ~0.5 us sooner (pipeline fill)
                    exv = ex_sb.rearrange("p (h q) -> p h q", h=2)
                    scv = sc_ps.rearrange("p (h q) -> p h q", h=2)
                    if kj == 0:
                        nc.scalar.activation(
                            exv[:, :, 0:256], scv[:, :, 0:256], Exp,
                            scale=SCALE)
                        nc.scalar.activation(
                            exv[:, :, 256:512], scv[:, :, 256:512], Exp,
                            scale=SCALE)
                    else:
                        nc.scalar.activation(
                            exv[:, :, qo : qo + W], scv[:, :, qo : qo + W],
                            Exp, scale=SCALE)
                    if kj >= 4 * gq:  # diagonal chunk: zero out k > q
                        exv = ex_sb.rearrange("p (h q) -> p h q", h=2)
                        nc.vector.tensor_mul(
                            exv[:, :, qo : qo + 128],
                            exv[:, :, qo : qo + 128],
                            trimask_sb.rearrange("p (h q) -> p h q", h=2),
                        )
                    # att_q[q, d_aug] += ex[k, q]^T-contracted with v_aug
                    # (ex chunk is the stationary; LDWEIGHTS overlaps).
                    # One accumulation group per tile: PSUM groups are
                    # per-bank, so the sub-chunk column slices share a
                    # single start/stop window.
                    for h in range(2):
                        for c in range(max(0, kj - 4 * gq), 4):
                            nc.tensor.matmul(
                                att_ps[h][:, c * 65 : c * 65 + 65],
                                ex_sb[:, h * 512 + c * 128
                                      : h * 512 + (c + 1) * 128],
                                v2_sb[:, kj * 160 + h * 80
                                      : kj * 160 + h * 80 + 65],
                                start=(kj == 0 and c == 0),
                                stop=(kj == njk - 1 and c == 3),
                            )
                # normalize: stage PSUM->SBUF in one copy (frees the 'acc'
                # ring, which gates the next qkv groups, ~1 us sooner), then
                # per-partition reciprocal + scalar muls into attQ
                # [q, c*128 + h*64 + d] fp16
                for h in range(2):
                    st = sb.tile([128, 260], f32, tag=f"st{h}",
                                 name=f"st{h}")
                    nc.vector.tensor_copy(st, att_ps[h])
                    rr4 = sb.tile([128, 4], f32, tag=f"rr{h}", name=f"rr{h}")
                    nc.vector.reciprocal(
                        rr4, st.rearrange("p (c t) -> p c t", t=65)[:, :, 64]
                    )
                    for c in range(4):
                        nc.vector.tensor_scalar_mul(
                            attQ_sb[:, gq * 512 + c * 128 + h * 64
                                    : gq * 512 + c * 128 + h * 64 + 64],
                            st[:, c * 65 : c * 65 + 64],
                            rr4[:, c : c + 1],
                        )
                # (the attQ->attT transpose and attB copies are emitted
                # later via emit_attn_finish, so they never park ahead of
                # the next v2 transpose on the SP ring)

            def emit_attn_finish(attQ_sb, attT_sb, attB, gq, after=()):
                # transpose attQ block -> attT [h*64+d, q] via the DMA xbar.
                # `after`: nosync deps forcing the scheduler to place this
                # chain behind the given v2 transposes on the SP ring (it
                # would otherwise hoist it ahead of them, and the transpose
                # guard would stall PV on v2 for ~8 us).
                attT3 = attT_sb.rearrange("p (c t) -> p c t", t=128)
                ti = nc.sync.dma_start(
                    attT3[:, 4 * gq : 4 * gq + 4, :],
                    attQ_sb[:, gq * 512 : (gq + 1) * 512],
                    transpose=True,
                )
                if after:
                    deps = InstructionNameOrderedSet()
                    for bi in after:
                        deps.add(bi.ins.name)
                    ti.ins.add_nosync_dependencies_from(deps)
                # o_proj operand (attB cols h*S+q): top = attn_h[d, q],
                # bottom = attn_h[d, q+1] (shifted), so K=128 w-pair chunks
                # read with one stride-16 AP.  Block gq enables shifted-dest
                # columns [gq*512-1, gq*512+511).
                lo = gq * 512
                src_lo = max(1, lo)
                for h in range(2):
                    nc.sync.dma_start(
                        attB[0:64, h * S + lo : h * S + lo + 512],
                        attT_sb[h * 64 : (h + 1) * 64, lo : lo + 512],
                    )
                    nc.sync.dma_start(
                        attB[64:128, h * S + src_lo - 1 : h * S + lo + 511],
                        attT_sb[h * 64 : (h + 1) * 64, src_lo : lo + 512],
                    )

            def emit_oproj_group(b, attB, out_sbs, h, n2):
                # out_band[u, c] = sum_j sum_{k<128}
                #   attB[k, h*S + u*16+2j] Wo[128j+k, c]   (K=128 per matmul)
                attv = attB[:, h * S : (h + 1) * S].rearrange(
                    "p (u w) -> p w u", w=16)
                po = ps.tile([128, 512], f32, tag="acc", name="po", bufs=4)
                for j in range(8):
                    nc.tensor.matmul(
                        po,
                        attv[:, 2 * j, :],
                        wo8_sb[:, j * E + n2 * 512 : j * E + n2 * 512 + 512],
                        start=(j == 0),
                        stop=(j == 7),
                    )
                # bias folded into the PSUM drain on DVE (saves a 512-cycle
                # K=1 matmul per group on the bottleneck engine)
                nc.vector.tensor_add(
                    out_sbs[h][:, n2 * 512 : (n2 + 1) * 512], po,
                    bo2d_sb[:, n2 * 512 : (n2 + 1) * 512],
                )
                if n2 == 1:
                    nc.sync.dma_start(out.ap()[b, h], out_sbs[h])

            # schedule per batch: [qkv n0, qkv n1, gq0, qkv n2, gq1, qkv n3,
            # gq2, gq3] with o_proj(b-1) groups woven into the ACT-bound
            # gq2/gq3 regions so PE never drains while ACT catches up.
            prev = None
            fin_args = None
            for b in range(B):
                xt_sb = emit_xload(b, 0, 1)
                qkvT2_sb = sb.tile([128, 3 * S], f16, tag="qkvT2")
                # v2 chunk layout (stride 160):
                #   [v_h0(64) | ones | pad15 | v_h1(64) | ones | pad15]
                v2_sb = sb.tile([128, 160 * (S // 128)], f16, tag="v2")
                v2v = v2_sb.rearrange("p (c t) -> p c t", t=160)
                nc.gpsimd.memset(v2v[:, :, 64:65], 1.0)
                nc.gpsimd.memset(v2v[:, :, 144:145], 1.0)
                attQ_sb = sb.tile([128, S], f16, tag="attQ", name="attQ")
                attT_sb = sb.tile([128, S], f16, tag="attT", name="attT")
                attB = sb.tile([128, 2 * S], f16, tag="attB", name="attB")

                vts = {}

                def qkv_m(n, m, vt=False):
                    def fn():
                        emit_qkv_mgroup(xt_sb, qkvT2_sb, n, m)
                        if vt:
                            vts[n] = emit_vtrans(qkvT2_sb, v2v, n)
                    return fn

                def attn(gq, fillers=()):
                    emit_attn_gq(qkvT2_sb, v2_sb, attQ_sb, attT_sb, attB,
                                 gq, fillers)

                def finish(gq, after_n):
                    def fn():
                        emit_attn_finish(attQ_sb, attT_sb, attB, gq,
                                         after=vts.get(after_n, ()))
                    return fn

                # v-group first so the v2 transpose (2.2 us DMA latency)
                # overlaps the q/k groups instead of stalling gq0's PV
                emit_qkv_mgroup(xt_sb, qkvT2_sb, 0, 2)
                vts[0] = emit_vtrans(qkvT2_sb, v2v, 0)
                if b == 0:
                    emit_xload(0, 1, 4, xt_sb)  # rest of batch-0 x after vt0
                if fin_args is not None:
                    # deferred gq2-finish of b-1, ordered behind vt0
                    emit_attn_finish(*fin_args, 2, after=vts[0])
                emit_qkv_mgroup(xt_sb, qkvT2_sb, 0, 0)
                emit_qkv_mgroup(xt_sb, qkvT2_sb, 0, 1)
                emit_qkv_mgroup(xt_sb, qkvT2_sb, 1, 0)
                emit_qkv_mgroup(xt_sb, qkvT2_sb, 1, 1)
                # filler layout follows the 'acc' PSUM ring (4 bufs): at
                # most two pq/po groups inside each attention block (their
                # ring gates resolve pre-block), one right after it, rest in
                # the head.  finish() DMAs allocate no PSUM; each is nosync-
                # ordered behind the next v2-transpose pair so the scheduler
                # never parks its copy chain ahead of them on the SP ring.
                f0 = [(1, qkv_m(1, 2, vt=True)), (2, qkv_m(2, 0))]
                if fin_args is not None:
                    fa = fin_args
                    f0.append((3, lambda: emit_attn_finish(
                        *fa, 3, after=vts[1])))
                attn(0, f0)
                emit_qkv_mgroup(xt_sb, qkvT2_sb, 2, 1)  # post-gq0
                if b == 0:
                    emit_wo8(0, 8)  # ACT ring; emitted after batch 0's
                    # first attention block so the transfers never sit in
                    # flight ahead of the startup v2 transposes
                attn(1, [(2, qkv_m(2, 2, vt=True)), (5, qkv_m(3, 0)),
                         (6, finish(0, 2))])
                emit_qkv_mgroup(xt_sb, qkvT2_sb, 3, 1)  # post-gq1
                if prev is not None:
                    pb, pattB, pout = prev
                    attn(2, [(2, qkv_m(3, 2, vt=True)), (4, finish(1, 3)),
                             (5, lambda: emit_oproj_group(
                                 pb, pattB, pout, 0, 0))])
                    emit_oproj_group(pb, pattB, pout, 0, 1)  # post-gq2
                    attn(3, [(2, lambda: emit_oproj_group(
                                 pb, pattB, pout, 1, 0)),
                             (5, lambda: emit_oproj_group(
                                 pb, pattB, pout, 1, 1))])
                else:
                    attn(2, [(2, qkv_m(3, 2, vt=True)), (4, finish(1, 3))])
                    attn(3)
                out_sbs = [
                    sb.tile([128, E], f16, tag=f"outsb{h}", name=f"out{h}_sb")
                    for h in range(2)
                ]
                prev = (b, attB, out_sbs)
                fin_args = (attQ_sb, attT_sb, attB)
            pb, pattB, pout = prev
            emit_attn_finish(*fin_args, 2)
            emit_attn_finish(*fin_args, 3)
            for h in range(2):
                for n2 in range(2):
                    emit_oproj_group(pb, pattB, pout, h, n2)

    nc.compile()
    return nc


def _get_program(dbg=False):
    key = ("nc",)
    if key not in _CACHE:
        _CACHE[key] = _build_program()
    return _CACHE[key]


def _host_inputs(x, Wqkv, bqkv, Wo, bo):
    """Build per-core input maps (host-side layout prep: cast/slice/transpose)."""
    xT = np.ascontiguousarray(x.transpose(0, 2, 1)).astype(np.float16)

    wo8 = np.ascontiguousarray(
        Wo.astype(np.float16).reshape(8, 128, E)
    )
    bo1 = np.ascontiguousarray(
        np.broadcast_to(bo.astype(np.float16)[None, :], (128, E))
    )

    k_idx = np.arange(128)[:, None]
    q_idx = np.arange(128)[None, :]
    tri = (k_idx <= q_idx).astype(np.float16)
    trimask = np.concatenate([tri, tri], axis=1)  # one copy per head

    in_maps = []
    for c in range(NCORES):
        cols = []
        for off in (0, 64, 128):  # q, k, v
            for h in (HPC * c, HPC * c + 1):
                cols.extend(range(h * 3 * D + off, h * 3 * D + off + 64))
        cols = np.asarray(cols)
        in_maps.append(
            {
                "xT": xT,
                "wqkv": np.ascontiguousarray(Wqkv[:, cols]).astype(np.float16),
                "bqkv": np.ascontiguousarray(
                    bqkv[cols].reshape(3, 128).T
                ).astype(np.float32),
                "wo8": wo8,
                "bo1": bo1,
                "trimask": trimask,
            }
        )
    return in_maps


def kernel(x, mask, Wqkv, bqkv, Wo, bo, _n_cores=NCORES, _trace=False, _dbg=False):
    """Full-input, full-output MHA. `mask` is the causal tril mask (hardcoded)."""
    from concourse.bass_utils import run_bass_kernel_spmd

    nc = _get_program()
    in_maps = _host_inputs(
        np.asarray(x), np.asarray(Wqkv), np.asarray(bqkv), np.asarray(Wo), np.asarray(bo)
    )[:_n_cores]
    res = run_bass_kernel_spmd(
        nc, in_maps, core_ids=list(range(_n_cores)), trace=_trace
    )
    out_full = np.zeros((B, S, E), np.float32)
    for c in range(_n_cores):
        o = res.results[c]["out"]  # [B, HPC, 128, E]
        for h in range(HPC):
            g = HPC * c + h
            out_full[:, g * 128 : (g + 1) * 128, :] = o[:, h]
    _CACHE["last_results"] = res
    return out_full


# revision 94
# speedup vs baseline: 1.0244x; 1.0117x over previous
"""Trainium2 Bass kernel for nn_MultiHeadAttention_46093589021200.

Causal MHA: B=4, S=2048, E=1024, H=16, D=64, with the reference's
"no-transpose-back" reshape (b,h,s,d)->(b,s,e) before the output projection.

Sharding: pure head-parallel, 2 heads per core, zero collectives.
Because of the reshape quirk, output rows s' in [h*128,(h+1)*128) depend only
on head h, so each core produces two independent 128-row output bands per
batch.

Device algorithm (per core, fp16 compute / fp32 PSUM accumulation):
  - qkvT = Wqkv_c^T @ x^T computed directly in head-major [col, s] layout
    (x is passed pre-transposed+pre-cast from the host; contraction over e
    in 8 PSUM-accumulated K=128 chunks; v's bias-add drains on ACT, q/k on
    DVE).
  - v transposed to [s, d] via the DMA xbar (one batched transpose per
    (head, 512-col chunk)), augmented with a ones column per head so the PV
    matmul also produces softmax denominators.
  - scoresT[k,q] per 128-k chunk on PE, two heads packed into row groups
    0-1 / 2-3 of the systolic array (K=64 each).
  - exp on ACT, one instruction covering both heads per chunk
    (scale=1/sqrt(D) folded in); causality = skipping k>q chunks entirely
    plus a triangular fp16 mask multiply on diagonal chunks (both heads in
    one DVE op via a doubled mask).
  - PV in q-major form: att_q[q, d_aug] accumulates with the exp'd score
    chunk as the stationary operand and v_aug streaming (N=65 per matmul,
    half the streaming cost of d-major PV); rowsums land per-partition so
    the normalize is one PSUM->SBUF staging copy + reciprocal +
    tensor_scalar multiplies (no partition broadcast).
  - attn transposed back to [h*64+d, q] per 512-block via the DMA xbar,
    then copied into attB: partitions 0-63 straight, 64-127 shifted one
    column left, so o_proj contracts K=128 (w-pairs) with one stride-16 AP.
  - o_proj: per 128-row band, 8 K=128-chunk matmuls; the bias is folded
    into the DVE PSUM-drain (tensor_add against a host-broadcast bias
    tile); output stored fp16 (host casts to fp32).

Scheduling: qkv n-chunks, o_proj(b-1) groups, and attention gq-blocks are
interleaved so the ACT engine's exp stream (the local bottleneck in the
attention phase) stays hidden under PE work; filler placement follows the
4-buffer PSUM ring so no matmul group parks on an att_ps free.  All
finish (transpose+copy) chains carry nosync deps on the next v2-transpose
pair — the scheduler would otherwise hoist them and the transpose-vs-DMA
serialization guard would stall PV.  x loads ride the SWDGE (gpsimd) ring
for the same reason.

NOTES: column-positioned matmuls (tile_position=(0,32j)) and GPSIMD
PSUM reads mis-execute / fail verification on this hardware path even
though the cost model accepts them; PSUM accumulation groups are
per-bank, so column-sliced accumulation windows must share one
start/stop.
"""

import sys

if "/opt/trn_rl_repo" not in sys.path:
    sys.path.insert(0, "/opt/trn_rl_repo")

import numpy as np

B, S, E, H = 4, 2048, 1024, 16
D = E // H          # 64
NCORES = 8
HPC = H // NCORES   # heads per core = 2
COLS = 3 * HPC * D  # 384 qkv columns per core
SCALE = 1.0 / float(np.sqrt(D))

_CACHE = {}


def _build_program():
    import concourse.bass as bass  # noqa: F401
    import concourse.tile as tile
    from concourse import bacc, mybir
    from concourse.instruction_name_ordered_set import InstructionNameOrderedSet

    f16 = mybir.dt.float16
    f32 = mybir.dt.float32
    Exp = mybir.ActivationFunctionType.Exp

    nc = bacc.Bacc("TRN2", target_bir_lowering=False, debug=False)

    xT = nc.dram_tensor("xT", [B, E, S], f16, kind="ExternalInput")
    wqkv = nc.dram_tensor("wqkv", [E, COLS], f16, kind="ExternalInput")
    bqkv = nc.dram_tensor("bqkv", [128, 3], f32, kind="ExternalInput")
    wo8 = nc.dram_tensor("wo8", [8, 128, E], f16, kind="ExternalInput")
    bo1 = nc.dram_tensor("bo1", [128, E], f16, kind="ExternalInput")
    trimask = nc.dram_tensor("trimask", [128, 256], f16, kind="ExternalInput")
    out = nc.dram_tensor("out", [B, HPC, 128, E], f16, kind="ExternalOutput")

    with tile.TileContext(nc) as tc:
        with (
            tc.tile_pool(name="const", bufs=1) as cp,
            tc.tile_pool(name="sb", bufs=2) as sb,
            tc.tile_pool(name="sb3", bufs=3) as sb3,
            tc.tile_pool(name="ps", bufs=2, space="PSUM") as ps,
        ):
            # ---- constants resident in SBUF for the whole kernel ----
            # critical path (SP ring): wqkv, then batch-0 x slices
            wqkv_sb = cp.tile([128, 8 * COLS], f16)   # [p, ec*384+col]
            nc.sync.dma_start(
                wqkv_sb.rearrange("p (ec c) -> p ec c", ec=8),
                wqkv.ap().rearrange("(ec p) c -> p ec c", p=128),
            )
            # non-critical constants on the ACT HWDGE ring
            bqkv_sb = cp.tile([128, 3], f32)
            nc.scalar.dma_start(bqkv_sb, bqkv.ap())
            trimask_sb = cp.tile([128, 256], f16)  # two copies side by side
            nc.scalar.dma_start(trimask_sb, trimask.ap())
            bo2d_sb = cp.tile([128, E], f16)  # bias broadcast to all rows
            nc.scalar.dma_start(bo2d_sb, bo1.ap())
            # wo8 loads in per-j chunks, interleaved into batch 0's stream:
            # a long const DMA in flight would stall the transpose-vs-DMA
            # serialization guard ahead of the v2 transposes
            wo8_sb = cp.tile([128, 8 * E], f16)       # [p, j*1024+c]

            def emit_wo8(j0, j1):
                for j in range(j0, j1):
                    nc.scalar.dma_start(
                        wo8_sb[:, j * E : (j + 1) * E], wo8.ap()[j]
                    )

            ones_sb = cp.tile([1, 128], f16)
            nc.vector.memset(ones_sb, 1.0)

            def emit_xload(b, lo=0, hi=4, xt_sb=None):
                # b==0: n-chunk slices, staggered around vt0 so the
                # transpose guard only waits on slice 0
                if xt_sb is None:
                    xt_sb = sb.tile([128, 8 * S], f16, tag="xt")
                xt3 = xt_sb.rearrange("p (ec s) -> p ec s", ec=8)
                xd3 = xT.ap()[b].rearrange("(ec p) s -> p ec s", p=128)
                # on the SWDGE (gpsimd) ring: HWDGE x-load transfers would
                # make the transpose-serialization guard stall v2 transposes
                if b == 0:
                    for n in range(lo, hi):
                        nc.gpsimd.dma_start(
                            xt3[:, :, n * 512 : (n + 1) * 512],
                            xd3[:, :, n * 512 : (n + 1) * 512],
                        )
                else:
                    for ec in range(8):
                        nc.gpsimd.dma_start(xt3[:, ec], xd3[:, ec])
                return xt_sb

            def emit_qkv_mgroup(xt_sb, qkvT2_sb, n, m):
                # one 512-wide s-chunk of one of q/k/v:
                # m=0 -> [q_h0|q_h1], m=1 -> [k_h0|k_h1], m=2 -> v
                pq = ps.tile([128, 512], f32, tag="acc", name="pq", bufs=4)
                for ec in range(8):
                    nc.tensor.matmul(
                        pq,
                        wqkv_sb[:, ec * COLS + m * 128
                                : ec * COLS + (m + 1) * 128],
                        xt_sb[:, ec * S + n * 512 : ec * S + (n + 1) * 512],
                        start=(ec == 0),
                        stop=(ec == 7),
                    )
                # bias-add + PSUM->SBUF drain.  GPSIMD cannot read PSUM on
                # hardware.  Early v groups (n<=1) drain on ACT (idle during
                # the qkv head, keeps DVE short for the normalize chain);
                # later ones stay on DVE so they don't delay mid-attention
                # exps on ACT.
                dst = qkvT2_sb[:, m * S + n * 512 : m * S + (n + 1) * 512]
                if m == 2 and n <= 1:
                    nc.scalar.activation(
                        dst, pq, mybir.ActivationFunctionType.Identity,
                        bias=bqkv_sb[:, m : m + 1],
                    )
                else:
                    nc.vector.tensor_scalar_add(dst, pq, bqkv_sb[:, m : m + 1])

            def emit_vtrans(qkvT2_sb, v2v, n):
                # transpose vT2 [d2, s] -> [s, d] batched per (h, n-chunk)
                insts = []
                for h in range(2):
                    insts.append(nc.sync.dma_start(
                        v2v[:, 4 * n : 4 * n + 4, h * 80 : h * 80 + 64],
                        qkvT2_sb[h * 64 : (h + 1) * 64,
                                 2 * S + n * 512 : 2 * S + (n + 1) * 512],
                        transpose=True,
                    ))
                return insts

            def emit_attn_sx(qkvT2_sb, gq, kj):
                # scores + exp + diagonal mask for one (gq, kj) chunk;
                # returns the exp'd-scores tile for the PV stage
                q_lo = max(gq * 512, kj * 128)
                W = gq * 512 + 512 - q_lo
                qo = q_lo - gq * 512
                sc_ps = ps.tile([128, 1024], f32, tag="scores",
                                name="sc_ps")
                ex_sb = sb3.tile([128, 1024], f16, tag="expT",
                                 name="ex_sb")
                for h in range(2):
                    # scoresT[k, q] = kT-chunk contracted with qT
                    nc.tensor.matmul(
                        sc_ps[:, h * 512 + qo : h * 512 + qo + W],
                        qkvT2_sb[h * 64 : (h + 1) * 64,
                                 S + kj * 128 : S + (kj + 1) * 128],
                        qkvT2_sb[h * 64 : (h + 1) * 64, q_lo : q_lo + W],
                        start=True,
                        stop=True,
                        tile_position=(h * 64, 0),
                    )
                # exp over both heads in one ACT instruction; the first
                # chunk of each block is split in half so its first PV
                # sub-chunks start ~0.5 us sooner (pipeline fill)
                exv = ex_sb.rearrange("p (h q) -> p h q", h=2)
                scv = sc_ps.rearrange("p (h q) -> p h q", h=2)
                if kj == 0:
                    nc.scalar.activation(
                        exv[:, :, 0:256], scv[:, :, 0:256], Exp,
                        scale=SCALE)
                    nc.scalar.activation(
                        exv[:, :, 256:512], scv[:, :, 256:512], Exp,
                        scale=SCALE)
                else:
                    nc.scalar.activation(
                        exv[:, :, qo : qo + W], scv[:, :, qo : qo + W],
                        Exp, scale=SCALE)
                if kj >= 4 * gq:  # diagonal chunk: zero out k > q
                    nc.vector.tensor_mul(
                        exv[:, :, qo : qo + 128],
                        exv[:, :, qo : qo + 128],
                        trimask_sb.rearrange("p (h q) -> p h q", h=2),
                    )
                return ex_sb

            def emit_attn_gq(qkvT2_sb, v2_sb, attQ_sb, attT_sb, attB, gq,
                             fillers=(), pre_ex=None, prefetch=False):
                fillers = dict(fillers)
                njk = 4 * gq + 4
                # q-major PV accumulators: [q=128, sub*65 + (d|rowsum)],
                # one per head.  N=65 per PV matmul (vs W) halves the PE
                # streaming cost; rowsums land per-partition so the
                # normalize is reciprocal + tensor_scalar (no broadcast).
                att_ps = [
                    ps.tile([128, 260], f32, tag="acc",
                            name=f"att{h}_ps", bufs=4)
                    for h in range(2)
                ]
                for kj in range(njk):
                    if kj in fillers:
                        fillers[kj]()
                    if kj == 0 and pre_ex is not None:
                        ex_sb = pre_ex  # scores+exp prefetched last block
                    else:
                        ex_sb = emit_attn_sx(qkvT2_sb, gq, kj)
                    # att_q[q, d_aug] += ex[k, q]^T-contracted with v_aug
                    # (ex chunk is the stationary; LDWEIGHTS overlaps).
                    # One accumulation group per tile: PSUM groups are
                    # per-bank, so the sub-chunk column slices share a
                    # single start/stop window.
                    for h in range(2):
                        for c in range(max(0, kj - 4 * gq), 4):
                            nc.tensor.matmul(
                                att_ps[h][:, c * 65 : c * 65 + 65],
                                ex_sb[:, h * 512 + c * 128
                                      : h * 512 + (c + 1) * 128],
                                v2_sb[:, kj * 160 + h * 80
                                      : kj * 160 + h * 80 + 65],
                                start=(kj == 0 and c == 0),
                                stop=(kj == njk - 1 and c == 3),
                            )
                # prefetch next block's first scores+exp so ACT never idles
                # across the block boundary waiting for PE's first scores
                next_ex = (emit_attn_sx(qkvT2_sb, gq + 1, 0)
                           if prefetch else None)
                # normalize: stage PSUM->SBUF in one copy (frees the 'acc'
                # ring, which gates the next qkv groups, ~1 us sooner), then
                # per-partition reciprocal + scalar muls into attQ
                # [q, c*128 + h*64 + d] fp16
                for h in range(2):
                    st = sb.tile([128, 260], f32, tag=f"st{h}",
                                 name=f"st{h}")
                    nc.vector.tensor_copy(st, att_ps[h])
                    rr4 = sb.tile([128, 4], f32, tag=f"rr{h}", name=f"rr{h}")
                    nc.vector.reciprocal(
                        rr4, st.rearrange("p (c t) -> p c t", t=65)[:, :, 64]
                    )
                    for c in range(4):
                        nc.vector.tensor_scalar_mul(
                            attQ_sb[:, gq * 512 + c * 128 + h * 64
                                    : gq * 512 + c * 128 + h * 64 + 64],
                            st[:, c * 65 : c * 65 + 64],
                            rr4[:, c : c + 1],
                        )
                # (the attQ->attT transpose and attB copies are emitted
                # later via emit_attn_finish, so they never park ahead of
                # the next v2 transpose on the SP ring)
                return next_ex

            def emit_attn_finish(attQ_sb, attT_sb, attB, gq, after=()):
                # transpose attQ block -> attT [h*64+d, q] via the DMA xbar.
                # `after`: nosync deps forcing the scheduler to place this
                # chain behind the given v2 transposes on the SP ring (it
                # would otherwise hoist it ahead of them, and the transpose
                # guard would stall PV on v2 for ~8 us).
                attT3 = attT_sb.rearrange("p (c t) -> p c t", t=128)
                ti = nc.sync.dma_start(
                    attT3[:, 4 * gq : 4 * gq + 4, :],
                    attQ_sb[:, gq * 512 : (gq + 1) * 512],
                    transpose=True,
                )
                if after:
                    deps = InstructionNameOrderedSet()
                    for bi in after:
                        deps.add(bi.ins.name)
                    ti.ins.add_nosync_dependencies_from(deps)
                # o_proj operand (attB cols h*S+q): top = attn_h[d, q],
                # bottom = attn_h[d, q+1] (shifted), so K=128 w-pair chunks
                # read with one stride-16 AP.  Block gq enables shifted-dest
                # columns [gq*512-1, gq*512+511).
                lo = gq * 512
                src_lo = max(1, lo)
                for h in range(2):
                    nc.sync.dma_start(
                        attB[0:64, h * S + lo : h * S + lo + 512],
                        attT_sb[h * 64 : (h + 1) * 64, lo : lo + 512],
                    )
                    nc.sync.dma_start(
                        attB[64:128, h * S + src_lo - 1 : h * S + lo + 511],
                        attT_sb[h * 64 : (h + 1) * 64, src_lo : lo + 512],
                    )

            def emit_oproj_group(b, attB, out_sbs, h, n2):
                # out_band[u, c] = sum_j sum_{k<128}
                #   attB[k, h*S + u*16+2j] Wo[128j+k, c]   (K=128 per matmul)
                attv = attB[:, h * S : (h + 1) * S].rearrange(
                    "p (u w) -> p w u", w=16)
                po = ps.tile([128, 512], f32, tag="acc", name="po", bufs=4)
                for j in range(8):
                    nc.tensor.matmul(
                        po,
                        attv[:, 2 * j, :],
                        wo8_sb[:, j * E + n2 * 512 : j * E + n2 * 512 + 512],
                        start=(j == 0),
                        stop=(j == 7),
                    )
                # bias folded into the PSUM drain on DVE (saves a 512-cycle
                # K=1 matmul per group on the bottleneck engine)
                nc.vector.tensor_add(
                    out_sbs[h][:, n2 * 512 : (n2 + 1) * 512], po,
                    bo2d_sb[:, n2 * 512 : (n2 + 1) * 512],
                )
                if n2 == 1:
                    nc.sync.dma_start(out.ap()[b, h], out_sbs[h])

            # schedule per batch: [qkv n0, qkv n1, gq0, qkv n2, gq1, qkv n3,
            # gq2, gq3] with o_proj(b-1) groups woven into the ACT-bound
            # gq2/gq3 regions so PE never drains while ACT catches up.
            prev = None
            fin_args = None
            for b in range(B):
                xt_sb = emit_xload(b, 0, 1)
                qkvT2_sb = sb.tile([128, 3 * S], f16, tag="qkvT2")
                # v2 chunk layout (stride 160):
                #   [v_h0(64) | ones | pad15 | v_h1(64) | ones | pad15]
                v2_sb = sb.tile([128, 160 * (S // 128)], f16, tag="v2")
                v2v = v2_sb.rearrange("p (c t) -> p c t", t=160)
                nc.gpsimd.memset(v2v[:, :, 64:65], 1.0)
                nc.gpsimd.memset(v2v[:, :, 144:145], 1.0)
                attQ_sb = sb.tile([128, S], f16, tag="attQ", name="attQ")
                attT_sb = sb.tile([128, S], f16, tag="attT", name="attT")
                attB = sb.tile([128, 2 * S], f16, tag="attB", name="attB")

                vts = {}

                def qkv_m(n, m, vt=False):
                    def fn():
                        emit_qkv_mgroup(xt_sb, qkvT2_sb, n, m)
                        if vt:
                            vts[n] = emit_vtrans(qkvT2_sb, v2v, n)
                    return fn

                pre_ex = [None]

                def attn(gq, fillers=()):
                    pre_ex[0] = emit_attn_gq(
                        qkvT2_sb, v2_sb, attQ_sb, attT_sb, attB, gq,
                        fillers, pre_ex=pre_ex[0], prefetch=(gq < 3))

                def finish(gq, after_n):
                    def fn():
                        emit_attn_finish(attQ_sb, attT_sb, attB, gq,
                                         after=vts.get(after_n, ()))
                    return fn

                # v-group first so the v2 transpose (2.2 us DMA latency)
                # overlaps the q/k groups instead of stalling gq0's PV
                emit_qkv_mgroup(xt_sb, qkvT2_sb, 0, 2)
                vts[0] = emit_vtrans(qkvT2_sb, v2v, 0)
                if b == 0:
                    emit_xload(0, 1, 4, xt_sb)  # rest of batch-0 x after vt0
                if fin_args is not None:
                    # deferred gq2-finish of b-1, ordered behind vt0
                    emit_attn_finish(*fin_args, 2, after=vts[0])
                emit_qkv_mgroup(xt_sb, qkvT2_sb, 0, 0)
                emit_qkv_mgroup(xt_sb, qkvT2_sb, 0, 1)
                emit_qkv_mgroup(xt_sb, qkvT2_sb, 1, 0)
                emit_qkv_mgroup(xt_sb, qkvT2_sb, 1, 1)
                # filler layout follows the 'acc' PSUM ring (4 bufs): at
                # most two pq/po groups inside each attention block (their
                # ring gates resolve pre-block), one right after it, rest in
                # the head.  finish() DMAs allocate no PSUM; each is nosync-
                # ordered behind the next v2-transpose pair so the scheduler
                # never parks its copy chain ahead of them on the SP ring.
                f0 = [(1, qkv_m(1, 2, vt=True)), (2, qkv_m(2, 0))]
                if fin_args is not None:
                    fa = fin_args
                    f0.append((3, lambda: emit_attn_finish(
                        *fa, 3, after=vts[1])))
                attn(0, f0)
                emit_qkv_mgroup(xt_sb, qkvT2_sb, 2, 1)  # post-gq0
                if b == 0:
                    emit_wo8(0, 8)  # ACT ring; emitted after batch 0's
                    # first attention block so the transfers never sit in
                    # flight ahead of the startup v2 transposes
                attn(1, [(2, qkv_m(2, 2, vt=True)), (5, qkv_m(3, 0)),
                         (6, finish(0, 2))])
                emit_qkv_mgroup(xt_sb, qkvT2_sb, 3, 1)  # post-gq1
                if prev is not None:
                    pb, pattB, pout = prev
                    attn(2, [(2, qkv_m(3, 2, vt=True)), (4, finish(1, 3)),
                             (5, lambda: emit_oproj_group(
                                 pb, pattB, pout, 0, 0))])
                    emit_oproj_group(pb, pattB, pout, 0, 1)  # post-gq2
                    attn(3, [(2, lambda: emit_oproj_group(
                                 pb, pattB, pout, 1, 0)),
                             (5, lambda: emit_oproj_group(
                                 pb, pattB, pout, 1, 1))])
                else:
                    attn(2, [(2, qkv_m(3, 2, vt=True)), (4, finish(1, 3))])
                    attn(3)
                out_sbs = [
                    sb.tile([128, E], f16, tag=f"outsb{h}", name=f"out{h}_sb")
                    for h in range(2)
                ]
                prev = (b, attB, out_sbs)
                fin_args = (attQ_sb, attT_sb, attB)
            pb, pattB, pout = prev
            emit_attn_finish(*fin_args, 2)
            emit_attn_finish(*fin_args, 3)
            for h in range(2):
                for n2 in range(2):
                    emit_oproj_group(pb, pattB, pout, h, n2)

    nc.compile()
    return nc


def _get_program(dbg=False):
    key = ("nc",)
    if key not in _CACHE:
        _CACHE[key] = _build_program()
    return _CACHE[key]


def _host_inputs(x, Wqkv, bqkv, Wo, bo):
    """Build per-core input maps (host-side layout prep: cast/slice/transpose)."""
    xT = np.ascontiguousarray(x.transpose(0, 2, 1)).astype(np.float16)

    wo8 = np.ascontiguousarray(
        Wo.astype(np.float16).reshape(8, 128, E)
    )
    bo1 = np.ascontiguousarray(
        np.broadcast_to(bo.astype(np.float16)[None, :], (128, E))
    )

    k_idx = np.arange(128)[:, None]
    q_idx = np.arange(128)[None, :]
    tri = (k_idx <= q_idx).astype(np.float16)
    trimask = np.concatenate([tri, tri], axis=1)  # one copy per head

    in_maps = []
    for c in range(NCORES):
        cols = []
        for off in (0, 64, 128):  # q, k, v
            for h in (HPC * c, HPC * c + 1):
                cols.extend(range(h * 3 * D + off, h * 3 * D + off + 64))
        cols = np.asarray(cols)
        in_maps.append(
            {
                "xT": xT,
                "wqkv": np.ascontiguousarray(Wqkv[:, cols]).astype(np.float16),
                "bqkv": np.ascontiguousarray(
                    bqkv[cols].reshape(3, 128).T
                ).astype(np.float32),
                "wo8": wo8,
                "bo1": bo1,
                "trimask": trimask,
            }
        )
    return in_maps


def kernel(x, mask, Wqkv, bqkv, Wo, bo, _n_cores=NCORES, _trace=False, _dbg=False):
    """Full-input, full-output MHA. `mask` is the causal tril mask (hardcoded)."""
    from concourse.bass_utils import run_bass_kernel_spmd

    nc = _get_program()
    in_maps = _host_inputs(
        np.asarray(x), np.asarray(Wqkv), np.asarray(bqkv), np.asarray(Wo), np.asarray(bo)
    )[:_n_cores]
    res = run_bass_kernel_spmd(
        nc, in_maps, core_ids=list(range(_n_cores)), trace=_trace
    )
    out_full = np.zeros((B, S, E), np.float32)
    for c in range(_n_cores):
        o = res.results[c]["out"]  # [B, HPC, 128, E]
        for h in range(HPC):
            g = HPC * c + h
            out_full[:, g * 128 : (g + 1) * 128, :] = o[:, h]
    _CACHE["last_results"] = res
    return out_full


# revision 97
# speedup vs baseline: 1.0325x; 1.0079x over previous
"""Trainium2 Bass kernel for nn_MultiHeadAttention_46093589021200.

Causal MHA: B=4, S=2048, E=1024, H=16, D=64, with the reference's
"no-transpose-back" reshape (b,h,s,d)->(b,s,e) before the output projection.

Sharding: pure head-parallel, 2 heads per core, zero collectives.
Because of the reshape quirk, output rows s' in [h*128,(h+1)*128) depend only
on head h, so each core produces two independent 128-row output bands per
batch.

Device algorithm (per core, fp16 compute / fp32 PSUM accumulation):
  - qkvT = Wqkv_c^T @ x^T computed directly in head-major [col, s] layout
    (x is passed pre-transposed+pre-cast from the host; contraction over e
    in 8 PSUM-accumulated K=128 chunks; v's bias-add drains on ACT, q/k on
    DVE).
  - v transposed to [s, d] via the DMA xbar (one batched transpose per
    (head, 512-col chunk)), augmented with a ones column per head so the PV
    matmul also produces softmax denominators.
  - scoresT[k,q] per 128-k chunk on PE, two heads packed into row groups
    0-1 / 2-3 of the systolic array (K=64 each).
  - exp on ACT, one instruction covering both heads per chunk
    (scale=1/sqrt(D) folded in); causality = skipping k>q chunks entirely
    plus a triangular fp16 mask multiply on diagonal chunks (both heads in
    one DVE op via a doubled mask).
  - PV in q-major form: att_q[q, d_aug] accumulates with the exp'd score
    chunk as the stationary operand and v_aug streaming (N=65 per matmul,
    half the streaming cost of d-major PV); rowsums land per-partition so
    the normalize is one PSUM->SBUF staging copy + reciprocal +
    tensor_scalar multiplies (no partition broadcast).
  - attn transposed back to [h*64+d, q] per 512-block via the DMA xbar,
    then copied into attB: partitions 0-63 straight, 64-127 shifted one
    column left, so o_proj contracts K=128 (w-pairs) with one stride-16 AP.
  - o_proj: per 128-row band, 8 K=128-chunk matmuls; the bias is folded
    into the DVE PSUM-drain (tensor_add against a host-broadcast bias
    tile); output stored fp16 (host casts to fp32).

Scheduling: qkv n-chunks, o_proj(b-1) groups, and attention gq-blocks are
interleaved so the ACT engine's exp stream (the local bottleneck in the
attention phase) stays hidden under PE work; filler placement follows the
4-buffer PSUM ring so no matmul group parks on an att_ps free.  All
finish (transpose+copy) chains carry nosync deps on the next v2-transpose
pair — the scheduler would otherwise hoist them and the transpose-vs-DMA
serialization guard would stall PV.  x loads ride the SWDGE (gpsimd) ring
for the same reason.

NOTES: column-positioned matmuls (tile_position=(0,32j)) and GPSIMD
PSUM reads mis-execute / fail verification on this hardware path even
though the cost model accepts them; PSUM accumulation groups are
per-bank, so column-sliced accumulation windows must share one
start/stop.
"""

import sys

if "/opt/trn_rl_repo" not in sys.path:
    sys.path.insert(0, "/opt/trn_rl_repo")

import numpy as np

B, S, E, H = 4, 2048, 1024, 16
D = E // H          # 64
NCORES = 8
HPC = H // NCORES   # heads per core = 2
COLS = 3 * HPC * D  # 384 qkv columns per core
SCALE = 1.0 / float(np.sqrt(D))

_CACHE = {}


def _build_program():
    import concourse.bass as bass  # noqa: F401
    import concourse.tile as tile
    from concourse import bacc, mybir
    from concourse.instruction_name_ordered_set import InstructionNameOrderedSet

    f16 = mybir.dt.float16
    f32 = mybir.dt.float32
    Exp = mybir.ActivationFunctionType.Exp

    nc = bacc.Bacc("TRN2", target_bir_lowering=False, debug=False)

    xT = nc.dram_tensor("xT", [B, E, S], f16, kind="ExternalInput")
    wqkv = nc.dram_tensor("wqkv", [E, COLS], f16, kind="ExternalInput")
    bqkv = nc.dram_tensor("bqkv", [128, 3], f32, kind="ExternalInput")
    wo8 = nc.dram_tensor("wo8", [8, 128, E], f16, kind="ExternalInput")
    bo1 = nc.dram_tensor("bo1", [128, E], f16, kind="ExternalInput")
    trimask = nc.dram_tensor("trimask", [128, 256], f16, kind="ExternalInput")
    out = nc.dram_tensor("out", [B, HPC, 128, E], f16, kind="ExternalOutput")

    with tile.TileContext(nc) as tc:
        with (
            tc.tile_pool(name="const", bufs=1) as cp,
            tc.tile_pool(name="sb", bufs=2) as sb,
            tc.tile_pool(name="sb3", bufs=3) as sb3,
            tc.tile_pool(name="ps", bufs=2, space="PSUM") as ps,
        ):
            # ---- constants resident in SBUF for the whole kernel ----
            # critical path (SP ring): wqkv, then batch-0 x slices
            wqkv_sb = cp.tile([128, 8 * COLS], f16)   # [p, ec*384+col]
            nc.sync.dma_start(
                wqkv_sb.rearrange("p (ec c) -> p ec c", ec=8),
                wqkv.ap().rearrange("(ec p) c -> p ec c", p=128),
            )
            # non-critical constants on the ACT HWDGE ring
            bqkv_sb = cp.tile([128, 3], f32)
            nc.scalar.dma_start(bqkv_sb, bqkv.ap())
            trimask_sb = cp.tile([128, 256], f16)  # two copies side by side
            nc.scalar.dma_start(trimask_sb, trimask.ap())
            bo2d_sb = cp.tile([128, E], f16)  # bias broadcast to all rows
            nc.scalar.dma_start(bo2d_sb, bo1.ap())
            # wo8 loads in per-j chunks, interleaved into batch 0's stream:
            # a long const DMA in flight would stall the transpose-vs-DMA
            # serialization guard ahead of the v2 transposes
            wo8_sb = cp.tile([128, 8 * E], f16)       # [p, j*1024+c]

            def emit_wo8(j0, j1):
                for j in range(j0, j1):
                    nc.scalar.dma_start(
                        wo8_sb[:, j * E : (j + 1) * E], wo8.ap()[j]
                    )

            ones_sb = cp.tile([1, 128], f16)
            nc.vector.memset(ones_sb, 1.0)

            def emit_xload(b, lo=0, hi=4, xt_sb=None):
                # b==0: n-chunk slices, staggered around vt0 so the
                # transpose guard only waits on slice 0
                if xt_sb is None:
                    xt_sb = sb.tile([128, 8 * S], f16, tag="xt")
                xt3 = xt_sb.rearrange("p (ec s) -> p ec s", ec=8)
                xd3 = xT.ap()[b].rearrange("(ec p) s -> p ec s", p=128)
                # on the SWDGE (gpsimd) ring: HWDGE x-load transfers would
                # make the transpose-serialization guard stall v2 transposes
                if b == 0:
                    for n in range(lo, hi):
                        nc.gpsimd.dma_start(
                            xt3[:, :, n * 512 : (n + 1) * 512],
                            xd3[:, :, n * 512 : (n + 1) * 512],
                        )
                else:
                    for ec in range(8):
                        nc.gpsimd.dma_start(xt3[:, ec], xd3[:, ec])
                return xt_sb

            def emit_qkv_mgroup(xt_sb, qkvT2_sb, n, m):
                # one 512-wide s-chunk of one of q/k/v:
                # m=0 -> [q_h0|q_h1], m=1 -> [k_h0|k_h1], m=2 -> v
                pq = ps.tile([128, 512], f32, tag="acc", name="pq", bufs=4)
                for ec in range(8):
                    nc.tensor.matmul(
                        pq,
                        wqkv_sb[:, ec * COLS + m * 128
                                : ec * COLS + (m + 1) * 128],
                        xt_sb[:, ec * S + n * 512 : ec * S + (n + 1) * 512],
                        start=(ec == 0),
                        stop=(ec == 7),
                    )
                # bias-add + PSUM->SBUF drain.  GPSIMD cannot read PSUM on
                # hardware.  Early v groups (n<=1) drain on ACT (idle during
                # the qkv head, keeps DVE short for the normalize chain);
                # later ones stay on DVE so they don't delay mid-attention
                # exps on ACT.
                dst = qkvT2_sb[:, m * S + n * 512 : m * S + (n + 1) * 512]
                if m == 2 and n <= 1:
                    nc.scalar.activation(
                        dst, pq, mybir.ActivationFunctionType.Identity,
                        bias=bqkv_sb[:, m : m + 1],
                    )
                else:
                    nc.vector.tensor_scalar_add(dst, pq, bqkv_sb[:, m : m + 1])

            def emit_vtrans(qkvT2_sb, v2v, n):
                # transpose vT2 [d2, s] -> [s, d] batched per (h, n-chunk)
                insts = []
                for h in range(2):
                    insts.append(nc.sync.dma_start(
                        v2v[:, 4 * n : 4 * n + 4, h * 80 : h * 80 + 64],
                        qkvT2_sb[h * 64 : (h + 1) * 64,
                                 2 * S + n * 512 : 2 * S + (n + 1) * 512],
                        transpose=True,
                    ))
                return insts

            def emit_attn_sx(qkvT2_sb, gq, kj):
                # scores + exp + diagonal mask for one (gq, kj) chunk;
                # returns the exp'd-scores tile for the PV stage
                q_lo = max(gq * 512, kj * 128)
                W = gq * 512 + 512 - q_lo
                qo = q_lo - gq * 512
                sc_ps = ps.tile([128, 1024], f32, tag="scores",
                                name="sc_ps")
                ex_sb = sb3.tile([128, 1024], f16, tag="expT",
                                 name="ex_sb")
                for h in range(2):
                    # scoresT[k, q] = kT-chunk contracted with qT
                    nc.tensor.matmul(
                        sc_ps[:, h * 512 + qo : h * 512 + qo + W],
                        qkvT2_sb[h * 64 : (h + 1) * 64,
                                 S + kj * 128 : S + (kj + 1) * 128],
                        qkvT2_sb[h * 64 : (h + 1) * 64, q_lo : q_lo + W],
                        start=True,
                        stop=True,
                        tile_position=(h * 64, 0),
                    )
                # exp over both heads in one ACT instruction; the first
                # chunk of each block is split in half so its first PV
                # sub-chunks start ~0.5 us sooner (pipeline fill)
                exv = ex_sb.rearrange("p (h q) -> p h q", h=2)
                scv = sc_ps.rearrange("p (h q) -> p h q", h=2)
                if kj == 0:
                    nc.scalar.activation(
                        exv[:, :, 0:256], scv[:, :, 0:256], Exp,
                        scale=SCALE)
                    nc.scalar.activation(
                        exv[:, :, 256:512], scv[:, :, 256:512], Exp,
                        scale=SCALE)
                else:
                    nc.scalar.activation(
                        exv[:, :, qo : qo + W], scv[:, :, qo : qo + W],
                        Exp, scale=SCALE)
                if kj >= 4 * gq:  # diagonal chunk: zero out k > q
                    nc.vector.tensor_mul(
                        exv[:, :, qo : qo + 128],
                        exv[:, :, qo : qo + 128],
                        trimask_sb.rearrange("p (h q) -> p h q", h=2),
                    )
                return ex_sb

            def emit_attn_gq(qkvT2_sb, v2_sb, attQ_sb, attT_sb, attB, gq,
                             fillers=(), pre_ex=None, prefetch=False):
                fillers = dict(fillers)
                njk = 4 * gq + 4
                # q-major PV accumulators: [q=128, sub*65 + (d|rowsum)],
                # one per head.  N=65 per PV matmul (vs W) halves the PE
                # streaming cost; rowsums land per-partition so the
                # normalize is reciprocal + tensor_scalar (no broadcast).
                att_ps = [
                    ps.tile([128, 260], f32, tag="acc",
                            name=f"att{h}_ps", bufs=4)
                    for h in range(2)
                ]
                for kj in range(njk):
                    if kj in fillers:
                        fillers[kj]()
                    if kj == 0 and pre_ex is not None:
                        ex_sb = pre_ex  # scores+exp prefetched last block
                    else:
                        ex_sb = emit_attn_sx(qkvT2_sb, gq, kj)
                    # att_q[q, d_aug] += ex[k, q]^T-contracted with v_aug
                    # (ex chunk is the stationary; LDWEIGHTS overlaps).
                    # One accumulation group per tile: PSUM groups are
                    # per-bank, so the sub-chunk column slices share a
                    # single start/stop window.
                    for h in range(2):
                        for c in range(max(0, kj - 4 * gq), 4):
                            nc.tensor.matmul(
                                att_ps[h][:, c * 65 : c * 65 + 65],
                                ex_sb[:, h * 512 + c * 128
                                      : h * 512 + (c + 1) * 128],
                                v2_sb[:, kj * 160 + h * 80
                                      : kj * 160 + h * 80 + 65],
                                start=(kj == 0 and c == 0),
                                stop=(kj == njk - 1 and c == 3),
                            )
                # prefetch next block's first scores+exp so ACT never idles
                # across the block boundary waiting for PE's first scores
                next_ex = (emit_attn_sx(qkvT2_sb, gq + 1, 0)
                           if prefetch else None)
                # normalize: stage PSUM->SBUF in one copy (frees the 'acc'
                # ring, which gates the next qkv groups, ~1 us sooner), then
                # per-partition reciprocal + scalar muls into attQ
                # [q, c*128 + h*64 + d] fp16
                for h in range(2):
                    st = sb.tile([128, 260], f32, tag=f"st{h}",
                                 name=f"st{h}")
                    nc.vector.tensor_copy(st, att_ps[h])
                    rr4 = sb.tile([128, 4], f32, tag=f"rr{h}", name=f"rr{h}")
                    nc.vector.reciprocal(
                        rr4, st.rearrange("p (c t) -> p c t", t=65)[:, :, 64]
                    )
                    for c in range(4):
                        nc.vector.tensor_scalar_mul(
                            attQ_sb[:, gq * 512 + c * 128 + h * 64
                                    : gq * 512 + c * 128 + h * 64 + 64],
                            st[:, c * 65 : c * 65 + 64],
                            rr4[:, c : c + 1],
                        )
                # (the attQ->attT transpose and attB copies are emitted
                # later via emit_attn_finish, so they never park ahead of
                # the next v2 transpose on the SP ring)
                return next_ex

            def emit_attn_finish(attQ_sb, attT_sb, attB, gq, after=()):
                # transpose attQ block -> attT [h*64+d, q] via the DMA xbar.
                # `after`: nosync deps forcing the scheduler to place this
                # chain behind the given v2 transposes on the SP ring (it
                # would otherwise hoist it ahead of them, and the transpose
                # guard would stall PV on v2 for ~8 us).
                attT3 = attT_sb.rearrange("p (c t) -> p c t", t=128)
                ti = nc.sync.dma_start(
                    attT3[:, 4 * gq : 4 * gq + 4, :],
                    attQ_sb[:, gq * 512 : (gq + 1) * 512],
                    transpose=True,
                )
                if after:
                    deps = InstructionNameOrderedSet()
                    for bi in after:
                        deps.add(bi.ins.name)
                    ti.ins.add_nosync_dependencies_from(deps)
                # o_proj operand (attB cols h*S+q): top = attn_h[d, q],
                # bottom = attn_h[d, q+1] (shifted), so K=128 w-pair chunks
                # read with one stride-16 AP.  Block gq enables shifted-dest
                # columns [gq*512-1, gq*512+511).
                lo = gq * 512
                src_lo = max(1, lo)
                for h in range(2):
                    nc.sync.dma_start(
                        attB[0:64, h * S + lo : h * S + lo + 512],
                        attT_sb[h * 64 : (h + 1) * 64, lo : lo + 512],
                    )
                    nc.sync.dma_start(
                        attB[64:128, h * S + src_lo - 1 : h * S + lo + 511],
                        attT_sb[h * 64 : (h + 1) * 64, src_lo : lo + 512],
                    )

            def emit_oproj_group(b, attB, out_sbs, h, n2):
                # out_band[u, c] = sum_j sum_{k<128}
                #   attB[k, h*S + u*16+2j] Wo[128j+k, c]   (K=128 per matmul)
                attv = attB[:, h * S : (h + 1) * S].rearrange(
                    "p (u w) -> p w u", w=16)
                po = ps.tile([128, 512], f32, tag="acc", name="po", bufs=4)
                for j in range(8):
                    nc.tensor.matmul(
                        po,
                        attv[:, 2 * j, :],
                        wo8_sb[:, j * E + n2 * 512 : j * E + n2 * 512 + 512],
                        start=(j == 0),
                        stop=(j == 7),
                    )
                # bias folded into the PSUM drain on DVE (saves a 512-cycle
                # K=1 matmul per group on the bottleneck engine)
                nc.vector.tensor_add(
                    out_sbs[h][:, n2 * 512 : (n2 + 1) * 512], po,
                    bo2d_sb[:, n2 * 512 : (n2 + 1) * 512],
                )
                if n2 == 1:
                    nc.sync.dma_start(out.ap()[b, h], out_sbs[h])

            # schedule per batch: [qkv n0, qkv n1, gq0, qkv n2, gq1, qkv n3,
            # gq2, gq3] with o_proj(b-1) groups woven into the ACT-bound
            # gq2/gq3 regions so PE never drains while ACT catches up.
            prev = None
            fin_args = None
            for b in range(B):
                xt_sb = emit_xload(b, 0, 1)
                qkvT2_sb = sb.tile([128, 3 * S], f16, tag="qkvT2")
                # v2 chunk layout (stride 160):
                #   [v_h0(64) | ones | pad15 | v_h1(64) | ones | pad15]
                v2_sb = sb.tile([128, 160 * (S // 128)], f16, tag="v2")
                v2v = v2_sb.rearrange("p (c t) -> p c t", t=160)
                nc.gpsimd.memset(v2v[:, :, 64:65], 1.0)
                nc.gpsimd.memset(v2v[:, :, 144:145], 1.0)
                attQ_sb = sb.tile([128, S], f16, tag="attQ", name="attQ")
                attT_sb = sb.tile([128, S], f16, tag="attT", name="attT")
                attB = sb.tile([128, 2 * S], f16, tag="attB", name="attB")

                vts = {}

                def qkv_m(n, m, vt=False):
                    def fn():
                        emit_qkv_mgroup(xt_sb, qkvT2_sb, n, m)
                        if vt:
                            vts[n] = emit_vtrans(qkvT2_sb, v2v, n)
                    return fn

                pre_ex = [None]

                def attn(gq, fillers=()):
                    pre_ex[0] = emit_attn_gq(
                        qkvT2_sb, v2_sb, attQ_sb, attT_sb, attB, gq,
                        fillers, pre_ex=pre_ex[0], prefetch=(gq < 3))

                def finish(gq, after_n):
                    def fn():
                        emit_attn_finish(attQ_sb, attT_sb, attB, gq,
                                         after=vts.get(after_n, ()))
                    return fn

                # v-group first so the v2 transpose (2.2 us DMA latency)
                # overlaps the q/k groups instead of stalling gq0's PV
                emit_qkv_mgroup(xt_sb, qkvT2_sb, 0, 2)
                vts[0] = emit_vtrans(qkvT2_sb, v2v, 0)
                if b == 0:
                    emit_xload(0, 1, 4, xt_sb)  # rest of batch-0 x after vt0
                if fin_args is not None:
                    # deferred gq2-finish of b-1, ordered behind vt0
                    emit_attn_finish(*fin_args, 2, after=vts[0])
                emit_qkv_mgroup(xt_sb, qkvT2_sb, 0, 0)
                emit_qkv_mgroup(xt_sb, qkvT2_sb, 0, 1)
                # gq0's first scores+exp hoisted into the head: q/k of n0
                # are ready, and the exp runs on ACT under the remaining
                # head groups instead of stalling gq0's first PV
                pre_ex[0] = emit_attn_sx(qkvT2_sb, 0, 0)
                emit_qkv_mgroup(xt_sb, qkvT2_sb, 1, 0)
                emit_qkv_mgroup(xt_sb, qkvT2_sb, 1, 1)
                # filler layout follows the 'acc' PSUM ring (4 bufs): at
                # most two pq/po groups inside each attention block (their
                # ring gates resolve pre-block), one right after it, rest in
                # the head.  finish() DMAs allocate no PSUM; each is nosync-
                # ordered behind the next v2-transpose pair so the scheduler
                # never parks its copy chain ahead of them on the SP ring.
                f0 = [(1, qkv_m(1, 2, vt=True)), (2, qkv_m(2, 0))]
                if fin_args is not None:
                    fa = fin_args
                    f0.append((3, lambda: emit_attn_finish(
                        *fa, 3, after=vts[1])))
                attn(0, f0)
                emit_qkv_mgroup(xt_sb, qkvT2_sb, 2, 1)  # post-gq0
                if b == 0:
                    emit_wo8(0, 8)  # ACT ring; emitted after batch 0's
                    # first attention block so the transfers never sit in
                    # flight ahead of the startup v2 transposes
                attn(1, [(2, qkv_m(2, 2, vt=True)), (5, qkv_m(3, 0)),
                         (6, finish(0, 2))])
                emit_qkv_mgroup(xt_sb, qkvT2_sb, 3, 1)  # post-gq1
                if prev is not None:
                    pb, pattB, pout = prev
                    attn(2, [(2, qkv_m(3, 2, vt=True)), (4, finish(1, 3)),
                             (5, lambda: emit_oproj_group(
                                 pb, pattB, pout, 0, 0))])
                    emit_oproj_group(pb, pattB, pout, 0, 1)  # post-gq2
                    attn(3, [(2, lambda: emit_oproj_group(
                                 pb, pattB, pout, 1, 0)),
                             (5, lambda: emit_oproj_group(
                                 pb, pattB, pout, 1, 1))])
                else:
                    attn(2, [(2, qkv_m(3, 2, vt=True)), (4, finish(1, 3))])
                    attn(3)
                out_sbs = [
                    sb.tile([128, E], f16, tag=f"outsb{h}", name=f"out{h}_sb")
                    for h in range(2)
                ]
                prev = (b, attB, out_sbs)
                fin_args = (attQ_sb, attT_sb, attB)
            pb, pattB, pout = prev
            emit_attn_finish(*fin_args, 2)
            emit_attn_finish(*fin_args, 3)
            for h in range(2):
                for n2 in range(2):
                    emit_oproj_group(pb, pattB, pout, h, n2)

    nc.compile()
    return nc


def _get_program(dbg=False):
    key = ("nc",)
    if key not in _CACHE:
        _CACHE[key] = _build_program()
    return _CACHE[key]


def _host_inputs(x, Wqkv, bqkv, Wo, bo):
    """Build per-core input maps (host-side layout prep: cast/slice/transpose)."""
    xT = np.ascontiguousarray(x.transpose(0, 2, 1)).astype(np.float16)

    wo8 = np.ascontiguousarray(
        Wo.astype(np.float16).reshape(8, 128, E)
    )
    bo1 = np.ascontiguousarray(
        np.broadcast_to(bo.astype(np.float16)[None, :], (128, E))
    )

    k_idx = np.arange(128)[:, None]
    q_idx = np.arange(128)[None, :]
    tri = (k_idx <= q_idx).astype(np.float16)
    trimask = np.concatenate([tri, tri], axis=1)  # one copy per head

    in_maps = []
    for c in range(NCORES):
        cols = []
        for off in (0, 64, 128):  # q, k, v
            for h in (HPC * c, HPC * c + 1):
                cols.extend(range(h * 3 * D + off, h * 3 * D + off + 64))
        cols = np.asarray(cols)
        in_maps.append(
            {
                "xT": xT,
                "wqkv": np.ascontiguousarray(Wqkv[:, cols]).astype(np.float16),
                "bqkv": np.ascontiguousarray(
                    bqkv[cols].reshape(3, 128).T
                ).astype(np.float32),
                "wo8": wo8,
                "bo1": bo1,
                "trimask": trimask,
            }
        )
    return in_maps


def kernel(x, mask, Wqkv, bqkv, Wo, bo, _n_cores=NCORES, _trace=False, _dbg=False):
    """Full-input, full-output MHA. `mask` is the causal tril mask (hardcoded)."""
    from concourse.bass_utils import run_bass_kernel_spmd

    nc = _get_program()
    in_maps = _host_inputs(
        np.asarray(x), np.asarray(Wqkv), np.asarray(bqkv), np.asarray(Wo), np.asarray(bo)
    )[:_n_cores]
    res = run_bass_kernel_spmd(
        nc, in_maps, core_ids=list(range(_n_cores)), trace=_trace
    )
    out_full = np.zeros((B, S, E), np.float32)
    for c in range(_n_cores):
        o = res.results[c]["out"]  # [B, HPC, 128, E]
        for h in range(HPC):
            g = HPC * c + h
            out_full[:, g * 128 : (g + 1) * 128, :] = o[:, h]
    _CACHE["last_results"] = res
    return out_full
